# revision 7
# baseline (speedup 1.0000x reference)
"""Trainium2 Bass kernel for ByteSpectralEmbedding.

Math (mirrors the reference):
  signal = byte_ids/127.5 - 1                       [B, T]
  X[b,f] = DFT(signal)[:, :128]  (Xr, Xi)
  mag    = |X| * freq_bands
  feats  = [mag (t-invariant), sin(phase0 + 2pi*t*f/T)]   (cos tail truncated)
  h = feats @ w1 + b1 ; LayerNorm; *gamma+beta; gelu(exact); @ w2 + b2

Key identity used on device: with A = Xi/|X|, C = Xr/|X|,
  sin(phase0[b,f] + shift[t,f]) = A[b,f]*cos_s[t,f] + C[b,f]*sin_s[t,f]
so layer 1 becomes, per batch b,
  h[t,:] = cos_row(t) @ (diag(A_b) w1s) + sin_row(t) @ (diag(C_b) w1s) + cb_b
with cb_b = mag_b @ w1m + b1 (t-invariant, injected via a K=1 matmul).

Sharding: pure data parallel, 4 batch rows per core across 8 cores.
"""

import numpy as np

import concourse.bass as bass
import concourse.bacc as bacc
import concourse.mybir as mybir
import concourse.tile as tile
from concourse.bass_utils import run_bass_kernel_spmd

B, T, D = 32, 4096, 256
KF = 128          # frequency bins kept
H = 512           # hidden dim (2*D)
NCORES = 8
BL = B // NCORES  # batch rows per core
NTT = T // 128    # t-tiles per batch row
LN_EPS = 1e-5
MAGIC = 0x5F3759DF

f32 = mybir.dt.float32
f16 = mybir.dt.float16
i32 = mybir.dt.int32

_cache = {}


def _host_tables():
    """Input-independent trig tables and layouts (float32)."""
    t = np.arange(T, dtype=np.float64)
    f = np.arange(KF, dtype=np.float64)
    ang = (2.0 * np.pi / T) * np.outer(t, f)          # [T, KF]
    cos_tf = np.cos(ang)
    sin_tf = np.sin(ang)
    # DFT stationary tiles, [t, f] layout, p-swizzled: rows t = j*128+p
    trig_dft = np.concatenate([cos_tf, -sin_tf], axis=1)       # [T, 2KF]
    trig_sw = (
        trig_dft.reshape(NTT, 128, 2 * KF).transpose(1, 0, 2).reshape(128, -1)
    ).astype(np.float32)                                        # [128, 32*256]
    # L1 stationary tiles, [f, t] layout
    cos_ft = np.ascontiguousarray(cos_tf.T).astype(np.float32)  # [KF, T]
    sin_ft = np.ascontiguousarray(sin_tf.T).astype(np.float32)  # [KF, T]
    return trig_sw, cos_ft, sin_ft


def _build_program(gu_bu, g0, b0):
    """Build the Bass program. gu_bu: True if gamma/beta are uniform scalars
    (g0, b0 folded into the fused normalize+gelu ACT op)."""
    nc = bacc.Bacc("TRN2", debug=False)

    ids_d = nc.declare_dram_parameter("ids_sw", [128, NTT * BL], i32, isOutput=False)
    trig_d = nc.declare_dram_parameter("trig_sw", [128, NTT * 2 * KF], f32, isOutput=False)
    cosf_d = nc.declare_dram_parameter("cos_ft", [KF, T], f32, isOutput=False)
    sinf_d = nc.declare_dram_parameter("sin_ft", [KF, T], f32, isOutput=False)
    w1m_d = nc.declare_dram_parameter("w1m", [KF, H], f32, isOutput=False)
    w1s_d = nc.declare_dram_parameter("w1s", [KF, H], f32, isOutput=False)
    w2_d = nc.declare_dram_parameter("w2sw", [128, 4 * D], f16, isOutput=False)
    b1_d = nc.declare_dram_parameter("b1row", [1, H], f32, isOutput=False)
    b2_d = nc.declare_dram_parameter("b2row", [1, D], f32, isOutput=False)
    fb_d = nc.declare_dram_parameter("fbcol", [KF, 1], f32, isOutput=False)
    ones_d = nc.declare_dram_parameter("ones1", [1, 128], f32, isOutput=False)
    if not gu_bu:
        gam_d = nc.declare_dram_parameter("gam_bc", [128, H], f32, isOutput=False)
        bet_d = nc.declare_dram_parameter("bet_bc", [128, H], f32, isOutput=False)
    out_d = nc.declare_dram_parameter("out", [BL, T, D], f32, isOutput=True)

    with tile.TileContext(nc) as tc:
        with tc.tile_pool(name="const", bufs=1) as const:
            # ---- constants into SBUF
            s_trig = const.tile([128, NTT, 2 * KF], f32)
            nc.sync.dma_start(out=s_trig, in_=trig_d.ap().rearrange("p (j c) -> p j c", j=NTT))
            s_cos = const.tile([KF, T], f32)
            nc.sync.dma_start(out=s_cos, in_=cosf_d.ap())
            s_sin = const.tile([KF, T], f32)
            nc.sync.dma_start(out=s_sin, in_=sinf_d.ap())
            s_w1m = const.tile([KF, H], f32)
            nc.sync.dma_start(out=s_w1m, in_=w1m_d.ap())
            s_w1s = const.tile([KF, H], f32)
            nc.sync.dma_start(out=s_w1s, in_=w1s_d.ap())
            s_w2 = const.tile([128, 4, D], f16)
            nc.sync.dma_start(out=s_w2, in_=w2_d.ap().rearrange("p (c d) -> p c d", c=4))
            s_b1 = const.tile([1, H], f32)
            nc.sync.dma_start(out=s_b1, in_=b1_d.ap())
            s_b2 = const.tile([1, D], f32)
            nc.sync.dma_start(out=s_b2, in_=b2_d.ap())
            s_fb = const.tile([KF, 1], f32)
            nc.sync.dma_start(out=s_fb, in_=fb_d.ap())
            s_ones = const.tile([1, 128], f32)
            nc.sync.dma_start(out=s_ones, in_=ones_d.ap())
            s_ids = const.tile([128, NTT, BL], i32)
            nc.sync.dma_start(out=s_ids, in_=ids_d.ap().rearrange("p (j b) -> p j b", j=NTT))
            if not gu_bu:
                s_gam = const.tile([128, H], f32)
                nc.sync.dma_start(out=s_gam, in_=gam_d.ap())
                s_bet = const.tile([128, H], f32)
                nc.sync.dma_start(out=s_bet, in_=bet_d.ap())
            s_magic = const.tile([128, BL], i32)
            nc.vector.memset(s_magic, MAGIC)

            # per-batch L1 weights and cb rows, filled by the prologue
            s_wa = const.tile([KF, BL, H], f32)
            s_wc = const.tile([KF, BL, H], f32)
            s_cb = const.tile([1, BL, H], f32)

            # ---- prologue: DFT + per-batch weight prep
            with tc.tile_pool(name="pro_ps", bufs=1, space="PSUM") as pro_ps, \
                 tc.tile_pool(name="pro_sb", bufs=1) as pro_sb:
                # Warmup: one tiny matmul per DMA-loaded const the PE will read.
                # Walrus allows only ONE sync-wait on a fused Matmult+LDW; these
                # advance PE's vector clock past each DMA lane one at a time so
                # real matmuls never need two DMA waits at once.
                warm_ps = pro_ps.tile([1, 1], f32, name="warm_ps")
                warm16 = pro_ps.tile([1, 1], f32, name="warm16")
                for cst in (s_trig[0:1, 0, 0:1], s_ones[0:1, 0:1],
                            s_b1[0:1, 0:1], s_b2[0:1, 0:1],
                            s_w1m[0:1, 0:1], s_cos[0:1, 0:1], s_sin[0:1, 0:1]):
                    nc.tensor.matmul(warm_ps, cst, cst, start=True, stop=True)
                nc.tensor.matmul(warm16, s_w2[0:1, 0, 0:1], s_w2[0:1, 0, 0:1],
                                 start=True, stop=True)
                s_sig = pro_sb.tile([128, NTT, BL], f32)
                nc.vector.tensor_scalar(
                    out=s_sig, in0=s_ids, scalar1=1.0 / 127.5, scalar2=-1.0,
                    op0=mybir.AluOpType.mult, op1=mybir.AluOpType.add,
                )
                xr_ps = pro_ps.tile([KF, BL], f32)
                xi_ps = pro_ps.tile([KF, BL], f32)
                for j in range(NTT):
                    nc.tensor.matmul(
                        xr_ps, s_trig[:, j, 0:KF], s_sig[:, j, :],
                        start=(j == 0), stop=(j == NTT - 1),
                    )
                for j in range(NTT):
                    nc.tensor.matmul(
                        xi_ps, s_trig[:, j, KF : 2 * KF], s_sig[:, j, :],
                        start=(j == 0), stop=(j == NTT - 1),
                    )
                s_xr = pro_sb.tile([KF, BL], f32)
                nc.vector.tensor_copy(s_xr, xr_ps)
                s_xi = pro_sb.tile([KF, BL], f32)
                nc.vector.tensor_copy(s_xi, xi_ps)
                # mag0 = sqrt(xr^2 + xi^2)   (sqrt table set loads here, once)
                s_m2 = pro_sb.tile([KF, BL], f32)
                nc.vector.tensor_mul(s_m2, s_xr, s_xr)
                s_t = pro_sb.tile([KF, BL], f32)
                nc.vector.tensor_mul(s_t, s_xi, s_xi)
                nc.vector.tensor_add(s_m2, s_m2, s_t)
                s_mag0 = pro_sb.tile([KF, BL], f32)
                nc.scalar.activation(s_mag0, s_m2, mybir.ActivationFunctionType.Sqrt)
                s_inv = pro_sb.tile([KF, BL], f32)
                nc.vector.reciprocal(s_inv, s_mag0)
                s_A = pro_sb.tile([KF, BL], f32)
                nc.vector.tensor_mul(s_A, s_xi, s_inv)
                s_C = pro_sb.tile([KF, BL], f32)
                nc.vector.tensor_mul(s_C, s_xr, s_inv)
                s_mag = pro_sb.tile([KF, BL], f32)
                nc.vector.tensor_scalar_mul(s_mag, in0=s_mag0, scalar1=s_fb)
                # W_A[b] = diag(A_b) @ w1s ; W_C[b] = diag(C_b) @ w1s
                for b in range(BL):
                    nc.vector.tensor_scalar_mul(s_wa[:, b, :], in0=s_w1s, scalar1=s_A[:, b : b + 1])
                    nc.vector.tensor_scalar_mul(s_wc[:, b, :], in0=s_w1s, scalar1=s_C[:, b : b + 1])
                # cb = mag.T @ w1m + b1   -> [BL, H] in PSUM
                cb_ps = pro_ps.tile([BL, H], f32)
                nc.tensor.matmul(cb_ps, s_mag, s_w1m, start=True, stop=False)
                nc.tensor.matmul(cb_ps, s_ones[0:1, 0:BL], s_b1, start=False, stop=True)
                cb_sb = pro_sb.tile([BL, H], f32)
                nc.vector.tensor_copy(cb_sb, cb_ps)
                # stage cb rows at partition 0 (cross-partition move => DMA)
                for b in range(BL):
                    nc.sync.dma_start(out=s_cb[0:1, b, :], in_=cb_sb[b : b + 1, :])

            # One-time sync point: collapses all prologue-era engine waits so
            # main-loop matmuls never carry more than one sync-wait (walrus
            # allows a single wait on a fused Matmult+LDWEIGHTS).
            nc.all_engine_barrier()

            # ---- main loop
            with tc.tile_pool(name="ps_h", bufs=5, space="PSUM") as ps_h, \
                 tc.tile_pool(name="ps_o", bufs=3, space="PSUM") as ps_o, \
                 tc.tile_pool(name="work", bufs=3) as work:
                for tt in range(NTT):
                    tsl = bass.ts(tt, 128)
                    h_ps = [ps_h.tile([128, H], f32, tag="h", name=f"h_{tt}_{b}")
                            for b in range(BL)]
                    # L1: shared stationary across the 4 batch rows
                    for b in range(BL):
                        nc.tensor.matmul(h_ps[b], s_ones[0:1, 0:128], s_cb[0:1, b, :],
                                         start=True, stop=False)
                    for b in range(BL):
                        nc.tensor.matmul(h_ps[b], s_cos[:, tsl], s_wa[:, b, :],
                                         start=False, stop=False)
                    for b in range(BL):
                        nc.tensor.matmul(h_ps[b], s_sin[:, tsl], s_wc[:, b, :],
                                         start=False, stop=True)
                    # LayerNorm stats
                    mvb = work.tile([128, BL, 2], f32, tag="mv")
                    for b in range(BL):
                        st = work.tile([128, 6], f32, tag="bnst")
                        nc.vector.bn_stats(out=st, in_=h_ps[b])
                        nc.vector.bn_aggr(out=mvb[:, b, :], in_=st)
                    # rstd = rsqrt(var+eps): bit-trick init + 2 Newton steps (batched)
                    ve = work.tile([128, BL], f32, tag="ve")
                    nc.vector.tensor_scalar(out=ve, in0=mvb[:, :, 1], scalar1=LN_EPS,
                                            scalar2=None, op0=mybir.AluOpType.add)
                    sh = work.tile([128, BL], i32, tag="sh")
                    nc.vector.tensor_scalar(out=sh, in0=ve.bitcast(i32), scalar1=1,
                                            scalar2=None,
                                            op0=mybir.AluOpType.logical_shift_right)
                    y0 = work.tile([128, BL], f32, tag="y0")
                    nc.vector.tensor_tensor(out=y0.bitcast(i32), in0=s_magic, in1=sh,
                                            op=mybir.AluOpType.subtract)
                    t1 = work.tile([128, BL], f32, tag="t1")
                    for _ in range(2):
                        nc.vector.tensor_mul(t1, y0, y0)
                        nc.vector.tensor_mul(t1, t1, ve)
                        nc.vector.tensor_scalar(out=t1, in0=t1, scalar1=-0.5, scalar2=1.5,
                                                op0=mybir.AluOpType.mult,
                                                op1=mybir.AluOpType.add)
                        nc.vector.tensor_mul(y0, y0, t1)
                    # fused scale/bias for the ACT pass
                    scl = work.tile([128, BL], f32, tag="scl")
                    if gu_bu and g0 != 1.0:
                        nc.vector.tensor_scalar(out=scl, in0=y0, scalar1=g0, scalar2=None,
                                                op0=mybir.AluOpType.mult)
                    else:
                        nc.vector.tensor_copy(scl, y0)
                    bia = work.tile([128, BL], f32, tag="bia")
                    nc.vector.tensor_mul(bia, mvb[:, :, 0], scl)
                    nc.vector.tensor_scalar(out=bia, in0=bia, scalar1=-1.0,
                                            scalar2=(b0 if gu_bu else 0.0),
                                            op0=mybir.AluOpType.mult,
                                            op1=mybir.AluOpType.add)
                    for b in range(BL):
                        g_sb = work.tile([128, H], f16, tag="g")
                        if gu_bu:
                            nc.scalar.activation(
                                out=g_sb, in_=h_ps[b],
                                func=mybir.ActivationFunctionType.Gelu,
                                bias=bia[:, b : b + 1], scale=scl[:, b : b + 1],
                            )
                        else:
                            y_sb = work.tile([128, H], f32, tag="y")
                            nc.scalar.activation(
                                out=y_sb, in_=h_ps[b],
                                func=mybir.ActivationFunctionType.Identity,
                                bias=bia[:, b : b + 1], scale=scl[:, b : b + 1],
                            )
                            nc.vector.tensor_mul(y_sb, y_sb, s_gam)
                            y2 = work.tile([128, H], f16, tag="y2")
                            nc.vector.tensor_add(y2, y_sb, s_bet)
                            nc.scalar.activation(
                                out=g_sb, in_=y2,
                                func=mybir.ActivationFunctionType.Gelu,
                            )
                        gT = work.tile([128, 4, 128], f16, tag="gT")
                        nc.sync.dma_start_transpose(out=gT, in_=g_sb)
                        o_ps = ps_o.tile([128, D], f32, tag="o")
                        nc.tensor.matmul(o_ps, s_ones[0:1, 0:128], s_b2,
                                         start=True, stop=False)
                        for c in range(4):
                            nc.tensor.matmul(o_ps, gT[:, c, :], s_w2[:, c, :],
                                             start=False, stop=(c == 3))
                        o_sb = work.tile([128, D], f32, tag="osb")
                        nc.scalar.copy(out=o_sb, in_=o_ps)
                        nc.sync.dma_start(out=out_d.ap()[b, tsl, :], in_=o_sb)
            ctx_ps_o.__exit__(None, None, None)
            ctx_ps_h.__exit__(None, None, None)
    nc.finalize()
    return nc


def kernel(byte_ids, freq_bands, w1, b1, gamma, beta, w2, b2):
    byte_ids = np.asarray(byte_ids)
    freq_bands = np.asarray(freq_bands, dtype=np.float32)
    w1 = np.asarray(w1, dtype=np.float32)
    b1 = np.asarray(b1, dtype=np.float32)
    gamma = np.asarray(gamma, dtype=np.float32)
    beta = np.asarray(beta, dtype=np.float32)
    w2 = np.asarray(w2, dtype=np.float32)
    b2 = np.asarray(b2, dtype=np.float32)

    gu_bu = bool(np.all(gamma == gamma[0]) and np.all(beta == beta[0]))
    g0 = float(gamma[0])
    b0 = float(beta[0])

    key = (gu_bu, g0, b0)
    if key not in _cache:
        _cache[key] = _build_program(gu_bu, g0, b0)
    nc = _cache[key]

    trig_sw, cos_ft, sin_ft = _host_tables()
    w2sw = np.ascontiguousarray(
        w2.reshape(4, 128, D).transpose(1, 0, 2).reshape(128, 4 * D)
    ).astype(np.float16)

    shared = {
        "trig_sw": trig_sw,
        "cos_ft": cos_ft,
        "sin_ft": sin_ft,
        "w1m": np.ascontiguousarray(w1[:KF, :]),
        "w1s": np.ascontiguousarray(w1[KF : 2 * KF, :]),
        "w2sw": w2sw,
        "b1row": b1.reshape(1, H),
        "b2row": b2.reshape(1, D),
        "fbcol": freq_bands[:KF].reshape(KF, 1),
        "ones1": np.ones((1, 128), np.float32),
    }
    if not gu_bu:
        shared["gam_bc"] = np.broadcast_to(gamma.reshape(1, H), (128, H)).copy()
        shared["bet_bc"] = np.broadcast_to(beta.reshape(1, H), (128, H)).copy()

    in_maps = []
    for c in range(NCORES):
        ids_c = byte_ids[c * BL : (c + 1) * BL, :]           # [BL, T]
        ids_sw = np.ascontiguousarray(
            ids_c.T.reshape(NTT, 128, BL).transpose(1, 0, 2).reshape(128, NTT * BL)
        ).astype(np.int32)
        m = dict(shared)
        m["ids_sw"] = ids_sw
        in_maps.append(m)

    res = run_bass_kernel_spmd(nc, in_maps, list(range(NCORES)))
    outs = [res.results[c]["out"] for c in range(NCORES)]
    return np.concatenate(outs, axis=0).astype(np.float32)
